# revision 20
# baseline (speedup 1.0000x reference)
"""Trainium2 Bass kernel for nn_ActorCritic (GIN message passing + heads).

Strategy (8-core SPMD, graph-parallel):
  - Nodes partitioned contiguously across 8 cores (6250 each). Each core owns
    the edges whose *destination* lands in its node range (host-side bucketing).
  - Per GIN layer: node features h live replicated in HBM (node-major f32
    table). Each core dma_gather's h[src] for its edges (512B rows), builds
    per-tile one-hot dst matrices on DVE (iota == dstoff), and scatter-adds via
    TensorE matmul accumulation into per-window PSUM:  agg^T = X^T @ S.
  - Dense GIN MLPs run feat-major (weights as lhsT, h as moving tensor).
  - After each layer the core's 6250 new rows are PE-transposed to node-major
    and AllGather'd to rebuild the replicated table.
  - Per-graph mean pool via matmul with batch one-hot + AllReduce; the tiny
    heads are computed redundantly on every core; core 0's output is used.

The Bass program is JIT-specialized to the actual edge distribution (per-window
tile counts are compile-time constants = max over cores, with dummy-edge
padding: gather idx 0 / dstoff -1 -> all-zero one-hot row contributes nothing).
"""

import os
import sys

import ml_dtypes
import ml_dtypes
import numpy as np

for _p in ("/opt/trn_rl_repo",):
    if os.path.isdir(_p) and _p not in sys.path:
        sys.path.insert(0, _p)

from contextlib import ExitStack

import concourse.bacc as bacc
import concourse.mybir as mybir
import concourse.tile as tile
from bass_rust import AP
from concourse import bass_utils

F32 = mybir.dt.float32
BF16 = mybir.dt.bfloat16
I16 = mybir.dt.int16
RELU = mybir.ActivationFunctionType.Relu
EQ = mybir.AluOpType.is_equal
MUL = mybir.AluOpType.mult
ADD = mybir.AluOpType.add

# problem constants (hardcoded per contest contract)
N_NODES = 50000
N_GRAPHS = 64
NODE_DIM = 8
GLOB_DIM = 4
HID = 128
N_LAYERS = 3
N_HEAD_OUT = 16  # 6 destroy + 9 repair + 1 value rows, stacked

# tunables
N_CORES = 8
WIN = 128          # dst window per scatter-matmul group
CH = 7             # gather tiles per dma_gather call. HARD LIMIT: the SWDGE
                   # per-DMA descriptor ring holds 64 descs; a gather needs
                   # num_idxs/16+1 slots, so CH*128/16+1 must stay < 64
                   # (CH=12/24 deadlock the device; CH<=7 is safe)
NSWQ = 4           # SWDGE queues; gather calls round-robin across them
OB = 8             # one-hot tiles generated per DVE op
SPLIT = 32768      # int16 gather index split point

TRACE = False       # set by test harness to capture a profile
LAST_RESULT = None  # BassKernelResults of the last run

EDGE_BF16 = True    # bf16 tables/gather/one-hot/scatter-matmul (rel err
                    # ~5e-5). NOTE: never cast-DMA via SWDGE for the shard
                    # writeback — >64 descs/DMA overflows the ring; the bf16
                    # staging copy happens on DVE instead.

EDGE_BF16 = True    # bf16 tables/gather/one-hot/scatter-matmul (rel err
                    # ~5e-5). NOTE: never cast-DMA via SWDGE for the shard
                    # writeback — >64 descs/DMA overflows the ring; the bf16
                    # staging copy happens on DVE instead.

# debug bisect knobs
SKIP_SCATTER = False   # replace gather/scatter agg with agg=0
LAYERS_EFF = N_LAYERS  # number of GIN layers actually built


def _cdiv(a, b):
    return -(-a // b)


# --------------------------------------------------------------------------
# host-side preprocessing
# --------------------------------------------------------------------------

def _preprocess_edges(src, dst, n_nodes, n_cores, win, split):
    """Bucket/pad edges per core. Returns (T, per_core): T[w*2+half] is the
    compile-time tile count for that (window, half-table) and per_core holds
    the padded idx/dstoff arrays in stream layout."""
    n_per = n_nodes // n_cores
    n_win = _cdiv(n_per, win)
    nk = n_win * 2
    counts = np.zeros((n_cores, nk), np.int64)
    segs = []
    for c in range(n_cores):
        m = (dst >= c * n_per) & (dst < (c + 1) * n_per)
        s = src[m]
        dl = dst[m] - c * n_per
        key = (dl // win) * 2 + (s >= split)
        order = np.lexsort((s, key))
        s, dl, key = s[order], dl[order], key[order]
        counts[c] = np.bincount(key, minlength=nk)
        segs.append((s, dl % win))
    T = _cdiv(counts.max(0), 128)  # [nk]
    Tlo_tot = int(T[0::2].sum())
    Thi_tot = int(T[1::2].sum())
    T_tot = Tlo_tot + Thi_tot

    per_core = []
    for c in range(n_cores):
        s, doff = segs[c]
        bounds = np.concatenate([[0], np.cumsum(counts[c])])
        idx_lo = np.zeros(max(Tlo_tot, 1) * 128, np.int64)
        idx_hi = np.zeros(max(Thi_tot, 1) * 128, np.int64)
        dof = np.full(T_tot * 128, -1.0, np.float32)
        lo_pos = hi_pos = g_pos = 0
        for w in range(n_win):
            for half in (0, 1):
                k = w * 2 + half
                n = int(counts[c, k])
                Tt = int(T[k])
                seg_s = s[bounds[k]:bounds[k + 1]]
                seg_d = doff[bounds[k]:bounds[k + 1]]
                if half == 0:
                    idx_lo[lo_pos * 128: lo_pos * 128 + n] = seg_s
                    lo_pos += Tt
                else:
                    idx_hi[hi_pos * 128: hi_pos * 128 + n] = seg_s - split
                    hi_pos += Tt
                dof[g_pos * 128: g_pos * 128 + n] = seg_d
                g_pos += Tt

        def wrap16(a):
            v = a.reshape(-1, 16).T.astype(np.int16)  # [16, L/16]
            return np.ascontiguousarray(np.tile(v, (8, 1)))  # [128, L/16]

        per_core.append(dict(
            idlo=wrap16(idx_lo),
            idhi=wrap16(idx_hi),
            doff=np.ascontiguousarray(dof.reshape(-1, 128).T),  # [128, T_tot]
        ))
    return T, per_core


# --------------------------------------------------------------------------
# kernel builder
# --------------------------------------------------------------------------

def _bcast(ap, pat):
    """Raw AP on ap's tensor/offset: partition row of `ap` + given free rows."""
    return AP(ap.tensor, ap.offset, [list(ap.ap[0])] + [list(p) for p in pat])


class _GatherStream:
    """Streams edge tiles of one half-table through double-buffered slabs."""

    def __init__(self, nc, pool, idx_sb, table_ap, total_tiles, ch, name,
                 edt=F32, qrr=None):
        self.nc = nc
        self.edt = edt
        self.qrr = qrr if qrr is not None else [0]
        self.pool = pool
        self.idx_sb = idx_sb
        self.table_ap = table_ap
        self.total = total_tiles
        self.ch = ch
        self.name = name
        self.loaded = 0
        self.slabs = []  # (start_tile, n_tiles, slab_ap)
        self.pos = 0

    def _issue(self):
        n = min(self.ch, self.total - self.loaded)
        assert n > 0
        slab = self.pool.tile([128, self.ch, HID], self.edt,
                              tag=f"slab_{self.name}")
        nidx = n * 128
        q = self.qrr[0]
        self.qrr[0] = (q + 1) % NSWQ
        self.nc.gpsimd.dma_gather(
            out_ap=slab[:, 0:n, :],
            in_ap=self.table_ap,
            idxs_ap=self.idx_sb[:, self.loaded * 8:(self.loaded + n) * 8],
            num_idxs=nidx,
            num_idxs_reg=nidx,
            elem_size=HID,
            queue_num=q,
        )
        self.slabs.append((self.loaded, n, slab))
        self.loaded += n

    def tile_ap(self, p):
        while p >= self.loaded:
            self._issue()
        for start, n, slab in reversed(self.slabs):
            if start <= p < start + n:
                return slab[:, p - start, :]
        raise AssertionError("tile fell out of stream window")


def _build_nc(T, eps_vals, n_nodes, n_cores):
    n_per = n_nodes // n_cores
    n_win = _cdiv(n_per, WIN)
    n_pad = n_win * 128
    assert WIN == 128
    Tlo = [int(x) for x in T[0::2]]
    Thi = [int(x) for x in T[1::2]]
    Tlo_tot = sum(Tlo)
    Thi_tot = sum(Thi)
    T_tot = Tlo_tot + Thi_tot
    split = min(SPLIT, n_nodes)
    hi_rows = n_nodes - split  # may be 0
    G = N_GRAPHS
    EDT = BF16 if EDGE_BF16 else F32
    EDT = BF16 if EDGE_BF16 else F32

    nc = bacc.Bacc("TRN2", target_bir_lowering=False, debug=False,
                   num_devices=n_cores, num_swdge_queues=NSWQ)

    # ---- I/O ----
    def inp(name, shape, dt=F32):
        return nc.dram_tensor(name, shape, dt, kind="ExternalInput")

    d_xT = inp("xT", [NODE_DIM, n_pad])
    d_gT = inp("gT", [GLOB_DIM, G])
    d_batchf = inp("batchf", [128, n_win])
    d_invc = inp("invc", [128, G])
    d_iota = inp("iota", [128, WIN])
    d_iotae = inp("iotae", [128, WIN], BF16 if EDGE_BF16 else F32)
    d_iotae = inp("iotae", [128, WIN], BF16 if EDGE_BF16 else F32)
    d_ident = inp("ident", [128, 128])
    d_idlo = inp("idlo", [128, max(Tlo_tot, 1) * 8], I16)
    d_idhi = inp("idhi", [128, max(Thi_tot, 1) * 8], I16)
    d_doff = inp("doff", [128, T_tot], BF16 if EDGE_BF16 else F32)
    d_Win = inp("W_in", [NODE_DIM, HID])
    d_bin = inp("b_in", [128, 1])
    d_WG1 = inp("WG1", [N_LAYERS, HID, HID])
    d_WG2 = inp("WG2", [N_LAYERS, HID, HID])
    d_B1 = inp("B1", [128, N_LAYERS])
    d_B2 = inp("B2", [128, N_LAYERS])
    d_Wg = inp("Wg", [GLOB_DIM, HID])
    d_bg = inp("bg", [128, 1])
    d_Wc = inp("Wc", [2 * HID, HID])
    d_bc = inp("bc", [128, 1])
    d_WH1 = inp("WH1", [3, HID, HID])
    d_BH1 = inp("BH1", [128, 3])
    d_WH2 = inp("WH2", [HID, N_HEAD_OUT])
    d_BH2 = inp("BH2", [N_HEAD_OUT, 3])

    d_out = nc.dram_tensor("heads", [N_HEAD_OUT, G], F32, kind="ExternalOutput")

    # ---- internal DRAM ----
    d_hshard = nc.dram_tensor("hshard", [n_per, HID],
                              BF16 if EDGE_BF16 else F32, kind="Internal")
    # NOTE: gather tables must be Local — dma_gather from a Shared
    # addr-space tensor hard-crashes the exec unit (probed on HW).
    d_htab = [nc.dram_tensor(f"htab{i}", [n_nodes, HID],
                             BF16 if EDGE_BF16 else F32, kind="Internal")
              for i in range(LAYERS_EFF)]
    d_pool_in = nc.dram_tensor("pool_in", [128, G], F32, kind="Internal")
    d_pool_out = nc.dram_tensor("pool_out", [128, G], F32, kind="Internal",
                                addr_space="Shared")
    rg = [list(range(n_cores))]

    with tile.TileContext(nc) as tc, ExitStack() as ctx:
        const = ctx.enter_context(tc.tile_pool(name="const", bufs=1))
        work = ctx.enter_context(tc.tile_pool(name="work", bufs=1))
        gpool = ctx.enter_context(tc.tile_pool(name="gather", bufs=6))
        spool = ctx.enter_context(tc.tile_pool(name="onehot", bufs=4))
        xpool = ctx.enter_context(tc.tile_pool(name="xin", bufs=2))
        small = ctx.enter_context(tc.tile_pool(name="small", bufs=2))
        pssc = ctx.enter_context(tc.tile_pool(name="pssc", bufs=4, space="PSUM"))
        psmm = ctx.enter_context(tc.tile_pool(name="psmm", bufs=2, space="PSUM"))
        pstr = ctx.enter_context(tc.tile_pool(name="pstr", bufs=2, space="PSUM"))

        def load_const(dram, shape, dt=F32):
            t = const.tile(shape, dt, tag=dram.name)
            nc.sync.dma_start(t, dram.ap())
            return t

        iota_sb = load_const(d_iota, [128, WIN])
        iotae_sb = load_const(d_iotae, [128, WIN], BF16 if EDGE_BF16 else F32)
        iotae_sb = load_const(d_iotae, [128, WIN], BF16 if EDGE_BF16 else F32)
        ident_sb = load_const(d_ident, [128, 128])
        batchf_sb = load_const(d_batchf, [128, n_win])
        invc_sb = load_const(d_invc, [128, G])
        idlo_sb = load_const(d_idlo, [128, max(Tlo_tot, 1) * 8], I16)
        idhi_sb = load_const(d_idhi, [128, max(Thi_tot, 1) * 8], I16)
        doff_sb = load_const(d_doff, [128, T_tot],
                             BF16 if EDGE_BF16 else F32)
        Win_sb = load_const(d_Win, [NODE_DIM, HID])
        bin_sb = load_const(d_bin, [128, 1])
        B1_sb = load_const(d_B1, [128, N_LAYERS])
        B2_sb = load_const(d_B2, [128, N_LAYERS])
        Wg_sb = load_const(d_Wg, [GLOB_DIM, HID])
        bg_sb = load_const(d_bg, [128, 1])
        Wc_lo = const.tile([HID, HID], F32, tag="Wc_lo")
        nc.sync.dma_start(Wc_lo, d_Wc.ap()[0:HID, :])
        Wc_hi = const.tile([HID, HID], F32, tag="Wc_hi")
        nc.sync.dma_start(Wc_hi, d_Wc.ap()[HID:2 * HID, :])
        bc_sb = load_const(d_bc, [128, 1])
        BH1_sb = load_const(d_BH1, [128, 3])
        WH2_sb = load_const(d_WH2, [HID, N_HEAD_OUT])
        BH2_sb = load_const(d_BH2, [N_HEAD_OUT, 3])
        gT_sb = load_const(d_gT, [GLOB_DIM, G])

        WG1_sb = const.tile([128, N_LAYERS * HID], F32, tag="WG1")
        WG2_sb = const.tile([128, N_LAYERS * HID], F32, tag="WG2")
        WH1_sb = const.tile([128, 3 * HID], F32, tag="WH1")
        for l in range(N_LAYERS):
            nc.sync.dma_start(WG1_sb[:, l * HID:(l + 1) * HID],
                              d_WG1.ap()[l, :, :])
            nc.sync.dma_start(WG2_sb[:, l * HID:(l + 1) * HID],
                              d_WG2.ap()[l, :, :])
        for l in range(3):
            nc.sync.dma_start(WH1_sb[:, l * HID:(l + 1) * HID],
                              d_WH1.ap()[l, :, :])

        h_sb = work.tile([128, n_pad], F32, tag="h")       # feat-major own nodes
        mt_sb = work.tile([128, n_pad], F32, tag="mt")     # m, then t (in place)
        hn_sb = work.tile([128, n_pad], F32, tag="hnode")  # node-major staging
        if EDGE_BF16:
            hne_sb = work.tile([128, n_pad], BF16, tag="hnode_e")
        else:
            hne_sb = hn_sb  # same slab when edge dtype is f32

        # MLP column chunking
        chunks = []
        o = 0
        while o < n_pad:
            nw = min(512, n_pad - o)
            chunks.append((o, nw))
            o += nw

        # ---------- input projection ----------
        for (o, nw) in chunks:
            xt = xpool.tile([NODE_DIM, 512], F32, tag="xt")
            nc.sync.dma_start(xt[:, 0:nw], d_xT.ap()[:, o:o + nw])
            ps = psmm.tile([128, 512], F32, tag="psmm")
            nc.tensor.matmul(ps[:, 0:nw], lhsT=Win_sb, rhs=xt[:, 0:nw],
                             start=True, stop=True)
            nc.scalar.activation(h_sb[:, o:o + nw], ps[:, 0:nw], RELU,
                                 bias=bin_sb[:, 0:1])

        # ---------- node-major writeback / AllGather ----------
        def writeback(table, need_f32_hn=False):
            for i in range(n_win):
                pt = pstr.tile([128, 128], F32, tag="pstr")
                nc.tensor.transpose(pt, h_sb[:, i * 128:(i + 1) * 128],
                                    ident_sb)
                if table is not None:
                    nc.vector.tensor_copy(hne_sb[:, i * 128:(i + 1) * 128], pt)
                if table is None or need_f32_hn or hne_sb is hn_sb:
                    if hne_sb is not hn_sb or table is None:
                        nc.vector.tensor_copy(hn_sb[:, i * 128:(i + 1) * 128],
                                              pt)
            if table is None:
                return
            nfull = n_per // 128
            rem = n_per - nfull * 128
            # DRAM walk (p, i, f) to match SBUF walk (part, tile, feat)
            out_ap = AP(d_hshard.ap().tensor, 0,
                        [[HID, 128], [HID * 128, nfull], [1, HID]])
            in_ap = hne_sb[:, 0:nfull * 128].rearrange("p (i f) -> p i f",
                                                       f=HID)
            nc.sync.dma_start(out_ap, in_ap)
            if rem:
                nc.sync.dma_start(
                    d_hshard.ap()[nfull * 128:n_per, :],
                    hne_sb[0:rem, nfull * 128:nfull * 128 + HID])
            nc.gpsimd.collective_compute(
                "AllGather", mybir.AluOpType.bypass, replica_groups=rg,
                ins=[d_hshard.ap()], outs=[table.ap()])

        writeback(d_htab[0])

        # ---------- GIN layers ----------
        for l in range(LAYERS_EFF):
            table = d_htab[l]
            tab_lo = table.ap()[0:split, :]
            qrr = [0]  # shared round-robin across both streams
            slo = _GatherStream(nc, gpool, idlo_sb, tab_lo, Tlo_tot, CH,
                                "lo", edt=EDT, qrr=qrr)
            if hi_rows > 0 and Thi_tot > 0:
                tab_hi = table.ap()[split:n_nodes, :]
                shi = _GatherStream(nc, gpool, idhi_sb, tab_hi, Thi_tot, CH,
                                    "hi", edt=EDT, qrr=qrr)
            else:
                shi = None
            scale = float(eps_vals[l]) + 1.0

            # batched one-hot producer over global tile index
            oh_slabs = {}

            def onehot_ap(g, oh_slabs=oh_slabs):
                b = g // OB
                if b not in oh_slabs:
                    g0 = b * OB
                    n = min(OB, T_tot - g0)
                    S = spool.tile([128, OB, WIN], EDT, tag="S")
                    io_b = _bcast(iotae_sb, [[0, n], [1, WIN]])
                    do_b = _bcast(doff_sb[:, g0:g0 + n], [[1, n], [0, WIN]])
                    nc.vector.tensor_tensor(S[:, 0:n, :], io_b, do_b, op=EQ)
                    oh_slabs[b] = S
                return oh_slabs[b][:, g - b * OB, :]

            g = 0
            for w in range(n_win):
                tot = Tlo[w] + Thi[w]
                wsl = slice(w * 128, (w + 1) * 128)
                if tot == 0 or SKIP_SCATTER:
                    nc.vector.tensor_scalar_mul(mt_sb[:, wsl], h_sb[:, wsl],
                                                scale)
                    continue
                ps = pssc.tile([128, WIN], F32, tag="pssc")
                k = 0
                for st, Tw in ((slo, Tlo[w]), (shi, Thi[w])):
                    for _ in range(Tw):
                        x_ap = st.tile_ap(st.pos)
                        st.pos += 1
                        S_ap = onehot_ap(g)
                        g += 1
                        nc.tensor.matmul(ps, lhsT=x_ap, rhs=S_ap,
                                         start=(k == 0), stop=(k == tot - 1))
                        k += 1
                nc.vector.scalar_tensor_tensor(
                    out=mt_sb[:, wsl], in0=h_sb[:, wsl], scalar=scale,
                    in1=ps, op0=MUL, op1=ADD)

            # MLP: t = relu(m @ W1 + b1); h = relu(t @ W2 + b2)
            for (o, nw) in chunks:
                ps1 = psmm.tile([128, 512], F32, tag="psmm")
                nc.tensor.matmul(ps1[:, 0:nw],
                                 lhsT=WG1_sb[:, l * HID:(l + 1) * HID],
                                 rhs=mt_sb[:, o:o + nw], start=True, stop=True)
                nc.scalar.activation(mt_sb[:, o:o + nw], ps1[:, 0:nw], RELU,
                                     bias=B1_sb[:, l:l + 1])
                ps2 = psmm.tile([128, 512], F32, tag="psmm")
                nc.tensor.matmul(ps2[:, 0:nw],
                                 lhsT=WG2_sb[:, l * HID:(l + 1) * HID],
                                 rhs=mt_sb[:, o:o + nw], start=True, stop=True)
                nc.scalar.activation(h_sb[:, o:o + nw], ps2[:, 0:nw], RELU,
                                     bias=B2_sb[:, l:l + 1])

            writeback(d_htab[l + 1] if l + 1 < LAYERS_EFF else None)

        # ---------- per-graph mean pool ----------
        psp = pssc.tile([128, G], F32, tag="pssc")
        for i in range(n_win):
            Sg = spool.tile([128, G], F32, tag="Sg")
            bat_b = _bcast(batchf_sb[:, i:i + 1], [[0, G]])
            nc.vector.tensor_tensor(Sg, iota_sb[:, 0:G], bat_b, op=EQ)
            nc.tensor.matmul(psp, lhsT=hn_sb[:, i * 128:(i + 1) * 128],
                             rhs=Sg, start=(i == 0), stop=(i == n_win - 1))
        poolp = small.tile([128, G], F32, tag="poolp")
        nc.vector.tensor_copy(poolp, psp)
        nc.sync.dma_start(d_pool_in.ap(), poolp)
        nc.gpsimd.collective_compute(
            "AllReduce", ADD, replica_groups=rg,
            ins=[d_pool_in.ap()], outs=[d_pool_out.ap()])
        pooled = small.tile([128, G], F32, tag="pooled")
        nc.sync.dma_start(pooled, d_pool_out.ap())
        nc.vector.tensor_mul(pooled, pooled, invc_sb)

        # ---------- combine + heads ----------
        psg = pssc.tile([128, G], F32, tag="pssc")
        nc.tensor.matmul(psg, lhsT=Wg_sb, rhs=gT_sb, start=True, stop=True)
        gact = small.tile([128, G], F32, tag="gact")
        nc.scalar.activation(gact, psg, RELU, bias=bg_sb[:, 0:1])

        pse = pssc.tile([128, G], F32, tag="pssc")
        nc.tensor.matmul(pse, lhsT=Wc_lo, rhs=pooled,
                         start=True, stop=False)
        nc.tensor.matmul(pse, lhsT=Wc_hi, rhs=gact,
                         start=False, stop=True)
        emb = small.tile([128, G], F32, tag="emb")
        nc.scalar.activation(emb, pse, RELU, bias=bc_sb[:, 0:1])

        head_rows = [(0, 6), (6, 9), (15, 1)]
        for hidx, (r0, rn) in enumerate(head_rows):
            ps1 = pssc.tile([128, G], F32, tag="pssc")
            nc.tensor.matmul(ps1, lhsT=WH1_sb[:, hidx * HID:(hidx + 1) * HID],
                             rhs=emb, start=True, stop=True)
            th = small.tile([128, G], F32, tag="th")
            nc.scalar.activation(th, ps1, RELU, bias=BH1_sb[:, hidx:hidx + 1])
            ps2 = pstr.tile([N_HEAD_OUT, G], F32, tag="pstr")
            nc.tensor.matmul(ps2[0:rn, :], lhsT=WH2_sb[:, r0:r0 + rn],
                             rhs=th, start=True, stop=True)
            hb = small.tile([N_HEAD_OUT, G], F32, tag="headsb")
            nc.vector.tensor_scalar_add(hb[0:rn, :], ps2[0:rn, :],
                                        BH2_sb[0:rn, hidx:hidx + 1])
            nc.sync.dma_start(d_out.ap()[r0:r0 + rn, :], hb[0:rn, :])

    nc.compile()
    return nc


# --------------------------------------------------------------------------
# public entry point
# --------------------------------------------------------------------------

_CACHE = {}


def _pack_bh2(inputs):
    out = np.zeros((N_HEAD_OUT, 3), np.float32)
    for j, k in enumerate(("bd2", "br2", "bv2")):
        b = np.asarray(inputs[k], np.float32).reshape(-1)
        out[: b.shape[0], j] = b
    return out


def _make_in_maps(inputs, per_core, n_nodes, n_cores):
    n_per = n_nodes // n_cores
    n_win = _cdiv(n_per, WIN)
    n_pad = n_win * 128
    G = N_GRAPHS

    def f32(a):
        return np.ascontiguousarray(np.asarray(a, np.float32))

    node_features = f32(inputs["node_features"])
    batch = np.asarray(inputs["batch"], np.int64)
    gfeat = f32(inputs["global_features"])

    counts = np.bincount(batch, minlength=G)[:G]
    invc = (1.0 / np.maximum(counts, 1.0)).astype(np.float32)
    INVC = np.ascontiguousarray(np.tile(invc[None, :], (128, 1)))
    IOTA = np.ascontiguousarray(
        np.tile(np.arange(WIN, dtype=np.float32)[None, :], (128, 1)))
    edt = ml_dtypes.bfloat16 if EDGE_BF16 else np.float32
    IOTAE = np.ascontiguousarray(IOTA.astype(edt))
    IDENT = np.eye(128, dtype=np.float32)

    shared = dict(
        gT=f32(gfeat.T),
        invc=INVC,
        iota=IOTA,
        iotae=IOTAE,
        ident=IDENT,
        W_in=f32(inputs["W_in"]),
        b_in=f32(inputs["b_in"]).reshape(128, 1),
        WG1=f32(inputs["gin_W1"]),
        WG2=f32(inputs["gin_W2"]),
        B1=f32(np.asarray(inputs["gin_b1"]).T),
        B2=f32(np.asarray(inputs["gin_b2"]).T),
        Wg=f32(inputs["Wg"]),
        bg=f32(inputs["bg"]).reshape(128, 1),
        Wc=f32(inputs["W_comb"]),
        bc=f32(inputs["b_comb"]).reshape(128, 1),
        WH1=f32(np.stack([np.asarray(inputs["Wd1"]),
                          np.asarray(inputs["Wr1"]),
                          np.asarray(inputs["Wv1"])])),
        BH1=f32(np.stack([np.asarray(inputs["bd1"]),
                          np.asarray(inputs["br1"]),
                          np.asarray(inputs["bv1"])]).T),
        WH2=f32(np.concatenate([np.asarray(inputs["Wd2"]),
                                np.asarray(inputs["Wr2"]),
                                np.asarray(inputs["Wv2"])], axis=1)),
        BH2=_pack_bh2(inputs),
    )
    assert shared["WH2"].shape == (HID, N_HEAD_OUT)
    assert shared["BH2"].shape == (N_HEAD_OUT, 3)

    in_maps = []
    for c in range(n_cores):
        xT = np.zeros((NODE_DIM, n_pad), np.float32)
        xT[:, :n_per] = node_features[c * n_per:(c + 1) * n_per].T
        bf = np.full(n_pad, -1.0, np.float32)
        bf[:n_per] = batch[c * n_per:(c + 1) * n_per]
        BATCHF = np.ascontiguousarray(bf.reshape(n_win, 128).T)
        m = dict(shared)
        m.update(
            xT=np.ascontiguousarray(xT),
            batchf=BATCHF,
            idlo=per_core[c]["idlo"],
            idhi=per_core[c]["idhi"],
            doff=np.ascontiguousarray(per_core[c]["doff"].astype(edt)),
        )
        in_maps.append(m)
    return in_maps


def kernel(**inputs):
    global LAST_RESULT
    edge_index = np.asarray(inputs["edge_index"], np.int64)
    src, dst = edge_index[0], edge_index[1]
    n_nodes = int(np.asarray(inputs["node_features"]).shape[0])
    eps_vals = np.asarray(inputs["gin_eps"], np.float32)

    T, per_core = _preprocess_edges(src, dst, n_nodes, N_CORES, WIN, SPLIT)
    key = (n_nodes, N_CORES, WIN, CH, EDGE_BF16, NSWQ,
           tuple(int(x) for x in T),
           tuple(float(e) for e in eps_vals))
    if key not in _CACHE:
        _CACHE[key] = _build_nc(T, eps_vals, n_nodes, N_CORES)
    nc = _CACHE[key]

    in_maps = _make_in_maps(inputs, per_core, n_nodes, N_CORES)
    res = bass_utils.run_bass_kernel_spmd(
        nc, in_maps, core_ids=list(range(N_CORES)), trace=TRACE)
    LAST_RESULT = res
    heads = res.results[0]["heads"]  # [16, G]
    destroy = np.ascontiguousarray(heads[0:6].T)
    repair = np.ascontiguousarray(heads[6:15].T)
    value = np.ascontiguousarray(heads[15:16].T)
    return destroy, repair, value


# revision 22
# speedup vs baseline: 1.2382x; 1.2382x over previous
"""Trainium2 Bass kernel for nn_ActorCritic (GIN message passing + heads).

Strategy (8-core SPMD, graph-parallel):
  - Nodes partitioned contiguously across 8 cores (6250 each). Each core owns
    the edges whose *destination* lands in its node range (host-side bucketing).
  - Per GIN layer: node features h live replicated in HBM (node-major f32
    table). Each core dma_gather's h[src] for its edges (512B rows), builds
    per-tile one-hot dst matrices on DVE (iota == dstoff), and scatter-adds via
    TensorE matmul accumulation into per-window PSUM:  agg^T = X^T @ S.
  - Dense GIN MLPs run feat-major (weights as lhsT, h as moving tensor).
  - After each layer the core's 6250 new rows are PE-transposed to node-major
    and AllGather'd to rebuild the replicated table.
  - Per-graph mean pool via matmul with batch one-hot + AllReduce; the tiny
    heads are computed redundantly on every core; core 0's output is used.

The Bass program is JIT-specialized to the actual edge distribution (per-window
tile counts are compile-time constants = max over cores, with dummy-edge
padding: gather idx 0 / dstoff -1 -> all-zero one-hot row contributes nothing).
"""

import os
import sys

import ml_dtypes
import ml_dtypes
import numpy as np

for _p in ("/opt/trn_rl_repo",):
    if os.path.isdir(_p) and _p not in sys.path:
        sys.path.insert(0, _p)

from contextlib import ExitStack

import concourse.bacc as bacc
import concourse.mybir as mybir
import concourse.tile as tile
from bass_rust import AP
from concourse import bass_utils

F32 = mybir.dt.float32
BF16 = mybir.dt.bfloat16
I16 = mybir.dt.int16
RELU = mybir.ActivationFunctionType.Relu
EQ = mybir.AluOpType.is_equal
MUL = mybir.AluOpType.mult
ADD = mybir.AluOpType.add

# problem constants (hardcoded per contest contract)
N_NODES = 50000
N_GRAPHS = 64
NODE_DIM = 8
GLOB_DIM = 4
HID = 128
N_LAYERS = 3
N_HEAD_OUT = 16  # 6 destroy + 9 repair + 1 value rows, stacked

# tunables
N_CORES = 8
WIN = 128          # dst window per scatter-matmul group
CH = 7             # gather tiles per dma_gather call. HARD LIMIT: the SWDGE
                   # per-DMA descriptor ring holds 64 descs; a gather needs
                   # num_idxs/16+1 slots, so CH*128/16+1 must stay < 64
                   # (CH=12/24 deadlock the device; CH<=7 is safe)
NSWQ = 4           # SWDGE queues; gather calls round-robin across them
OB = 8             # one-hot tiles generated per DVE op
SPLIT = 32768      # int16 gather index split point

TRACE = False       # set by test harness to capture a profile
LAST_RESULT = None  # BassKernelResults of the last run

EDGE_BF16 = True    # bf16 tables/gather/one-hot/scatter-matmul (rel err
                    # ~5e-5). NOTE: never cast-DMA via SWDGE for the shard
                    # writeback — >64 descs/DMA overflows the ring; the bf16
                    # staging copy happens on DVE instead.

EDGE_BF16 = True    # bf16 tables/gather/one-hot/scatter-matmul (rel err
                    # ~5e-5). NOTE: never cast-DMA via SWDGE for the shard
                    # writeback — >64 descs/DMA overflows the ring; the bf16
                    # staging copy happens on DVE instead.

# debug bisect knobs
SKIP_SCATTER = False   # replace gather/scatter agg with agg=0
LAYERS_EFF = N_LAYERS  # number of GIN layers actually built


def _cdiv(a, b):
    return -(-a // b)


# --------------------------------------------------------------------------
# host-side preprocessing
# --------------------------------------------------------------------------

def _half_size(n_per):
    """Tile-aligned (128) split of each core's shard into A/B halves."""
    return min(n_per, _cdiv(_cdiv(n_per, 2), 128) * 128)


def _preprocess_edges(src, dst, n_nodes, n_cores, win, split):
    """Bucket/pad edges per core. Returns (T, per_core): T[w*2+half] is the
    compile-time tile count for that (window, half-table) and per_core holds
    the padded idx/dstoff arrays in stream layout. half = which half-shard
    table (A/B) the edge's source row lives in."""
    n_per = n_nodes // n_cores
    hs = _half_size(n_per)
    n_win = _cdiv(n_per, win)
    nk = n_win * 2
    counts = np.zeros((n_cores, nk), np.int64)
    segs = []
    src_core = src // n_per
    src_off = src % n_per
    rowA = src_core * hs + src_off                 # valid where src_off < hs
    rowB = src_core * (n_per - hs) + (src_off - hs)
    for c in range(n_cores):
        m = (dst >= c * n_per) & (dst < (c + 1) * n_per)
        s = np.where(src_off[m] < hs, rowA[m], rowB[m])
        half_f = (src_off[m] >= hs).astype(np.int64)
        dl = dst[m] - c * n_per
        key = (dl // win) * 2 + half_f
        order = np.lexsort((s, key))
        s, dl, key = s[order], dl[order], key[order]
        counts[c] = np.bincount(key, minlength=nk)
        segs.append((s, dl % win))
    T = _cdiv(counts.max(0), 128)  # [nk]
    Tlo_tot = int(T[0::2].sum())
    Thi_tot = int(T[1::2].sum())
    T_tot = Tlo_tot + Thi_tot

    per_core = []
    for c in range(n_cores):
        s, doff = segs[c]
        bounds = np.concatenate([[0], np.cumsum(counts[c])])
        idx_lo = np.zeros(max(Tlo_tot, 1) * 128, np.int64)
        idx_hi = np.zeros(max(Thi_tot, 1) * 128, np.int64)
        dof = np.full(T_tot * 128, -1.0, np.float32)
        lo_pos = hi_pos = g_pos = 0
        for w in range(n_win):
            for half in (0, 1):
                k = w * 2 + half
                n = int(counts[c, k])
                Tt = int(T[k])
                seg_s = s[bounds[k]:bounds[k + 1]]
                seg_d = doff[bounds[k]:bounds[k + 1]]
                if half == 0:
                    idx_lo[lo_pos * 128: lo_pos * 128 + n] = seg_s
                    lo_pos += Tt
                else:
                    idx_hi[hi_pos * 128: hi_pos * 128 + n] = seg_s
                    hi_pos += Tt
                dof[g_pos * 128: g_pos * 128 + n] = seg_d
                g_pos += Tt

        def wrap16(a):
            v = a.reshape(-1, 16).T.astype(np.int16)  # [16, L/16]
            return np.ascontiguousarray(np.tile(v, (8, 1)))  # [128, L/16]

        per_core.append(dict(
            idlo=wrap16(idx_lo),
            idhi=wrap16(idx_hi),
            doff=np.ascontiguousarray(dof.reshape(-1, 128).T),  # [128, T_tot]
        ))
    return T, per_core


# --------------------------------------------------------------------------
# kernel builder
# --------------------------------------------------------------------------

def _bcast(ap, pat):
    """Raw AP on ap's tensor/offset: partition row of `ap` + given free rows."""
    return AP(ap.tensor, ap.offset, [list(ap.ap[0])] + [list(p) for p in pat])


class _GatherStream:
    """Streams edge tiles of one half-table through double-buffered slabs."""

    def __init__(self, nc, pool, idx_sb, table_ap, total_tiles, ch, name,
                 edt=F32, qrr=None):
        self.nc = nc
        self.edt = edt
        self.qrr = qrr if qrr is not None else [0]
        self.pool = pool
        self.idx_sb = idx_sb
        self.table_ap = table_ap
        self.total = total_tiles
        self.ch = ch
        self.name = name
        self.loaded = 0
        self.slabs = []  # (start_tile, n_tiles, slab_ap)
        self.pos = 0

    def _issue(self):
        n = min(self.ch, self.total - self.loaded)
        assert n > 0
        slab = self.pool.tile([128, self.ch, HID], self.edt,
                              tag=f"slab_{self.name}")
        nidx = n * 128
        q = self.qrr[0]
        self.qrr[0] = (q + 1) % NSWQ
        self.nc.gpsimd.dma_gather(
            out_ap=slab[:, 0:n, :],
            in_ap=self.table_ap,
            idxs_ap=self.idx_sb[:, self.loaded * 8:(self.loaded + n) * 8],
            num_idxs=nidx,
            num_idxs_reg=nidx,
            elem_size=HID,
            queue_num=q,
        )
        self.slabs.append((self.loaded, n, slab))
        self.loaded += n

    def tile_ap(self, p):
        while p >= self.loaded:
            self._issue()
        for start, n, slab in reversed(self.slabs):
            if start <= p < start + n:
                return slab[:, p - start, :]
        raise AssertionError("tile fell out of stream window")


def _build_nc(T, eps_vals, n_nodes, n_cores):
    n_per = n_nodes // n_cores
    n_win = _cdiv(n_per, WIN)
    n_pad = n_win * 128
    assert WIN == 128
    Tlo = [int(x) for x in T[0::2]]
    Thi = [int(x) for x in T[1::2]]
    Tlo_tot = sum(Tlo)
    Thi_tot = sum(Thi)
    T_tot = Tlo_tot + Thi_tot
    hs = _half_size(n_per)          # A half rows per core (tile-aligned)
    bs = n_per - hs                 # B half rows per core
    G = N_GRAPHS
    EDT = BF16 if EDGE_BF16 else F32
    EDT = BF16 if EDGE_BF16 else F32

    nc = bacc.Bacc("TRN2", target_bir_lowering=False, debug=False,
                   num_devices=n_cores, num_swdge_queues=NSWQ)

    # ---- I/O ----
    def inp(name, shape, dt=F32):
        return nc.dram_tensor(name, shape, dt, kind="ExternalInput")

    d_xT = inp("xT", [NODE_DIM, n_pad])
    d_gT = inp("gT", [GLOB_DIM, G])
    d_batchf = inp("batchf", [128, n_win])
    d_invc = inp("invc", [128, G])
    d_iota = inp("iota", [128, WIN])
    d_iotae = inp("iotae", [128, WIN], BF16 if EDGE_BF16 else F32)
    d_iotae = inp("iotae", [128, WIN], BF16 if EDGE_BF16 else F32)
    d_ident = inp("ident", [128, 128])
    d_idlo = inp("idlo", [128, max(Tlo_tot, 1) * 8], I16)
    d_idhi = inp("idhi", [128, max(Thi_tot, 1) * 8], I16)
    d_doff = inp("doff", [128, T_tot], BF16 if EDGE_BF16 else F32)
    d_Win = inp("W_in", [NODE_DIM, HID])
    d_bin = inp("b_in", [128, 1])
    d_WG1 = inp("WG1", [N_LAYERS, HID, HID])
    d_WG2 = inp("WG2", [N_LAYERS, HID, HID])
    d_B1 = inp("B1", [128, N_LAYERS])
    d_B2 = inp("B2", [128, N_LAYERS])
    d_Wg = inp("Wg", [GLOB_DIM, HID])
    d_bg = inp("bg", [128, 1])
    d_Wc = inp("Wc", [2 * HID, HID])
    d_bc = inp("bc", [128, 1])
    d_WH1 = inp("WH1", [3, HID, HID])
    d_BH1 = inp("BH1", [128, 3])
    d_WH2 = inp("WH2", [HID, N_HEAD_OUT])
    d_BH2 = inp("BH2", [N_HEAD_OUT, 3])

    d_out = nc.dram_tensor("heads", [N_HEAD_OUT, G], F32, kind="ExternalOutput")

    # ---- internal DRAM ----
    d_hshardA = nc.dram_tensor("hshardA", [hs, HID], EDT, kind="Internal")
    d_hshardB = nc.dram_tensor("hshardB", [bs, HID], EDT, kind="Internal")
    # NOTE: gather tables must be Local — dma_gather from a Shared
    # addr-space tensor hard-crashes the exec unit (probed on HW).
    d_htabA = [nc.dram_tensor(f"htabA{i}", [n_cores * hs, HID], EDT,
                              kind="Internal") for i in range(LAYERS_EFF)]
    d_htabB = [nc.dram_tensor(f"htabB{i}", [n_cores * bs, HID], EDT,
                              kind="Internal") for i in range(LAYERS_EFF)]
    d_pool_in = nc.dram_tensor("pool_in", [128, G], F32, kind="Internal")
    d_pool_out = nc.dram_tensor("pool_out", [128, G], F32, kind="Internal",
                                addr_space="Shared")
    rg = [list(range(n_cores))]

    with tile.TileContext(nc) as tc, ExitStack() as ctx:
        const = ctx.enter_context(tc.tile_pool(name="const", bufs=1))
        work = ctx.enter_context(tc.tile_pool(name="work", bufs=1))
        gpool = ctx.enter_context(tc.tile_pool(name="gather", bufs=6))
        spool = ctx.enter_context(tc.tile_pool(name="onehot", bufs=4))
        xpool = ctx.enter_context(tc.tile_pool(name="xin", bufs=2))
        small = ctx.enter_context(tc.tile_pool(name="small", bufs=2))
        pssc = ctx.enter_context(tc.tile_pool(name="pssc", bufs=4, space="PSUM"))
        psmm = ctx.enter_context(tc.tile_pool(name="psmm", bufs=2, space="PSUM"))
        pstr = ctx.enter_context(tc.tile_pool(name="pstr", bufs=2, space="PSUM"))

        def load_const(dram, shape, dt=F32):
            t = const.tile(shape, dt, tag=dram.name)
            nc.sync.dma_start(t, dram.ap())
            return t

        iota_sb = load_const(d_iota, [128, WIN])
        iotae_sb = load_const(d_iotae, [128, WIN], BF16 if EDGE_BF16 else F32)
        iotae_sb = load_const(d_iotae, [128, WIN], BF16 if EDGE_BF16 else F32)
        ident_sb = load_const(d_ident, [128, 128])
        batchf_sb = load_const(d_batchf, [128, n_win])
        invc_sb = load_const(d_invc, [128, G])
        idlo_sb = load_const(d_idlo, [128, max(Tlo_tot, 1) * 8], I16)
        idhi_sb = load_const(d_idhi, [128, max(Thi_tot, 1) * 8], I16)
        doff_sb = load_const(d_doff, [128, T_tot],
                             BF16 if EDGE_BF16 else F32)
        Win_sb = load_const(d_Win, [NODE_DIM, HID])
        bin_sb = load_const(d_bin, [128, 1])
        B1_sb = load_const(d_B1, [128, N_LAYERS])
        B2_sb = load_const(d_B2, [128, N_LAYERS])
        Wg_sb = load_const(d_Wg, [GLOB_DIM, HID])
        bg_sb = load_const(d_bg, [128, 1])
        Wc_lo = const.tile([HID, HID], F32, tag="Wc_lo")
        nc.sync.dma_start(Wc_lo, d_Wc.ap()[0:HID, :])
        Wc_hi = const.tile([HID, HID], F32, tag="Wc_hi")
        nc.sync.dma_start(Wc_hi, d_Wc.ap()[HID:2 * HID, :])
        bc_sb = load_const(d_bc, [128, 1])
        BH1_sb = load_const(d_BH1, [128, 3])
        WH2_sb = load_const(d_WH2, [HID, N_HEAD_OUT])
        BH2_sb = load_const(d_BH2, [N_HEAD_OUT, 3])
        gT_sb = load_const(d_gT, [GLOB_DIM, G])

        WG1_sb = const.tile([128, N_LAYERS * HID], F32, tag="WG1")
        WG2_sb = const.tile([128, N_LAYERS * HID], F32, tag="WG2")
        WH1_sb = const.tile([128, 3 * HID], F32, tag="WH1")
        for l in range(N_LAYERS):
            nc.sync.dma_start(WG1_sb[:, l * HID:(l + 1) * HID],
                              d_WG1.ap()[l, :, :])
            nc.sync.dma_start(WG2_sb[:, l * HID:(l + 1) * HID],
                              d_WG2.ap()[l, :, :])
        for l in range(3):
            nc.sync.dma_start(WH1_sb[:, l * HID:(l + 1) * HID],
                              d_WH1.ap()[l, :, :])

        h_sb = work.tile([128, n_pad], F32, tag="h")       # feat-major own nodes
        mt_sb = work.tile([128, n_pad], F32, tag="mt")     # m, then t (in place)
        hn_sb = work.tile([128, n_pad], F32, tag="hnode")  # node-major staging
        if EDGE_BF16:
            hne_sb = work.tile([128, n_pad], BF16, tag="hnode_e")
        else:
            hne_sb = hn_sb  # same slab when edge dtype is f32

        # MLP column chunking
        chunks = []
        o = 0
        while o < n_pad:
            nw = min(512, n_pad - o)
            chunks.append((o, nw))
            o += nw

        # ---------- input projection ----------
        for (o, nw) in chunks:
            xt = xpool.tile([NODE_DIM, 512], F32, tag="xt")
            nc.sync.dma_start(xt[:, 0:nw], d_xT.ap()[:, o:o + nw])
            ps = psmm.tile([128, 512], F32, tag="psmm")
            nc.tensor.matmul(ps[:, 0:nw], lhsT=Win_sb, rhs=xt[:, 0:nw],
                             start=True, stop=True)
            nc.scalar.activation(h_sb[:, o:o + nw], ps[:, 0:nw], RELU,
                                 bias=bin_sb[:, 0:1])

        # ---------- node-major writeback / split AllGather ----------
        tilesA = hs // 128

        def writeback(tables):
            last = tables is None
            for i in range(n_win):
                pt = pstr.tile([128, 128], F32, tag="pstr")
                nc.tensor.transpose(pt, h_sb[:, i * 128:(i + 1) * 128],
                                    ident_sb)
                if not last:
                    nc.vector.tensor_copy(hne_sb[:, i * 128:(i + 1) * 128], pt)
                if last or hne_sb is hn_sb:
                    if hne_sb is not hn_sb or last:
                        nc.vector.tensor_copy(hn_sb[:, i * 128:(i + 1) * 128],
                                              pt)
            if last:
                return
            tabA, tabB = tables
            # A half: tiles [0, tilesA) -> hshardA, AG-A fires once the first
            # half of the MLP + transposes is done (overlaps the rest).
            out_a = AP(d_hshardA.ap().tensor, 0,
                       [[HID, 128], [HID * 128, tilesA], [1, HID]])
            in_a = hne_sb[:, 0:tilesA * 128].rearrange("p (i f) -> p i f",
                                                       f=HID)
            nc.sync.dma_start(out_a, in_a)
            nc.gpsimd.collective_compute(
                "AllGather", mybir.AluOpType.bypass, replica_groups=rg,
                ins=[d_hshardA.ap()], outs=[tabA.ap()])
            # B half: remaining full tiles + ragged tail -> hshardB, AG-B.
            nfullB = bs // 128
            rem = bs - nfullB * 128
            if nfullB:
                out_b = AP(d_hshardB.ap().tensor, 0,
                           [[HID, 128], [HID * 128, nfullB], [1, HID]])
                in_b = hne_sb[:, tilesA * 128:(tilesA + nfullB) * 128]
                in_b = in_b.rearrange("p (i f) -> p i f", f=HID)
                nc.sync.dma_start(out_b, in_b)
            if rem:
                nc.sync.dma_start(
                    d_hshardB.ap()[nfullB * 128:bs, :],
                    hne_sb[0:rem, (tilesA + nfullB) * 128:
                           (tilesA + nfullB) * 128 + HID])
            nc.gpsimd.collective_compute(
                "AllGather", mybir.AluOpType.bypass, replica_groups=rg,
                ins=[d_hshardB.ap()], outs=[tabB.ap()])

        writeback((d_htabA[0], d_htabB[0]))

        # ---------- GIN layers ----------
        for l in range(LAYERS_EFF):
            qrr = [0]  # shared round-robin across both streams
            slo = _GatherStream(nc, gpool, idlo_sb, d_htabA[l].ap(), Tlo_tot,
                                CH, "lo", edt=EDT, qrr=qrr)
            if Thi_tot > 0:
                shi = _GatherStream(nc, gpool, idhi_sb, d_htabB[l].ap(),
                                    Thi_tot, CH, "hi", edt=EDT, qrr=qrr)
            else:
                shi = None
            scale = float(eps_vals[l]) + 1.0

            # batched one-hot producer over global tile index
            oh_slabs = {}

            def onehot_ap(g, oh_slabs=oh_slabs):
                b = g // OB
                if b not in oh_slabs:
                    g0 = b * OB
                    n = min(OB, T_tot - g0)
                    S = spool.tile([128, OB, WIN], EDT, tag="S")
                    io_b = _bcast(iotae_sb, [[0, n], [1, WIN]])
                    do_b = _bcast(doff_sb[:, g0:g0 + n], [[1, n], [0, WIN]])
                    nc.vector.tensor_tensor(S[:, 0:n, :], io_b, do_b, op=EQ)
                    oh_slabs[b] = S
                return oh_slabs[b][:, g - b * OB, :]

            g = 0
            for w in range(n_win):
                tot = Tlo[w] + Thi[w]
                wsl = slice(w * 128, (w + 1) * 128)
                if tot == 0 or SKIP_SCATTER:
                    nc.vector.tensor_scalar_mul(mt_sb[:, wsl], h_sb[:, wsl],
                                                scale)
                    continue
                ps = pssc.tile([128, WIN], F32, tag="pssc")
                k = 0
                for st, Tw in ((slo, Tlo[w]), (shi, Thi[w])):
                    for _ in range(Tw):
                        x_ap = st.tile_ap(st.pos)
                        st.pos += 1
                        S_ap = onehot_ap(g)
                        g += 1
                        nc.tensor.matmul(ps, lhsT=x_ap, rhs=S_ap,
                                         start=(k == 0), stop=(k == tot - 1))
                        k += 1
                nc.vector.scalar_tensor_tensor(
                    out=mt_sb[:, wsl], in0=h_sb[:, wsl], scalar=scale,
                    in1=ps, op0=MUL, op1=ADD)

            # MLP: t = relu(m @ W1 + b1); h = relu(t @ W2 + b2)
            for (o, nw) in chunks:
                ps1 = psmm.tile([128, 512], F32, tag="psmm")
                nc.tensor.matmul(ps1[:, 0:nw],
                                 lhsT=WG1_sb[:, l * HID:(l + 1) * HID],
                                 rhs=mt_sb[:, o:o + nw], start=True, stop=True)
                nc.scalar.activation(mt_sb[:, o:o + nw], ps1[:, 0:nw], RELU,
                                     bias=B1_sb[:, l:l + 1])
                ps2 = psmm.tile([128, 512], F32, tag="psmm")
                nc.tensor.matmul(ps2[:, 0:nw],
                                 lhsT=WG2_sb[:, l * HID:(l + 1) * HID],
                                 rhs=mt_sb[:, o:o + nw], start=True, stop=True)
                nc.scalar.activation(h_sb[:, o:o + nw], ps2[:, 0:nw], RELU,
                                     bias=B2_sb[:, l:l + 1])

            writeback((d_htabA[l + 1], d_htabB[l + 1])
                      if l + 1 < LAYERS_EFF else None)

        # ---------- per-graph mean pool ----------
        psp = pssc.tile([128, G], F32, tag="pssc")
        for i in range(n_win):
            Sg = spool.tile([128, G], F32, tag="Sg")
            bat_b = _bcast(batchf_sb[:, i:i + 1], [[0, G]])
            nc.vector.tensor_tensor(Sg, iota_sb[:, 0:G], bat_b, op=EQ)
            nc.tensor.matmul(psp, lhsT=hn_sb[:, i * 128:(i + 1) * 128],
                             rhs=Sg, start=(i == 0), stop=(i == n_win - 1))
        poolp = small.tile([128, G], F32, tag="poolp")
        nc.vector.tensor_copy(poolp, psp)
        nc.sync.dma_start(d_pool_in.ap(), poolp)
        nc.gpsimd.collective_compute(
            "AllReduce", ADD, replica_groups=rg,
            ins=[d_pool_in.ap()], outs=[d_pool_out.ap()])
        pooled = small.tile([128, G], F32, tag="pooled")
        nc.sync.dma_start(pooled, d_pool_out.ap())
        nc.vector.tensor_mul(pooled, pooled, invc_sb)

        # ---------- combine + heads ----------
        psg = pssc.tile([128, G], F32, tag="pssc")
        nc.tensor.matmul(psg, lhsT=Wg_sb, rhs=gT_sb, start=True, stop=True)
        gact = small.tile([128, G], F32, tag="gact")
        nc.scalar.activation(gact, psg, RELU, bias=bg_sb[:, 0:1])

        pse = pssc.tile([128, G], F32, tag="pssc")
        nc.tensor.matmul(pse, lhsT=Wc_lo, rhs=pooled,
                         start=True, stop=False)
        nc.tensor.matmul(pse, lhsT=Wc_hi, rhs=gact,
                         start=False, stop=True)
        emb = small.tile([128, G], F32, tag="emb")
        nc.scalar.activation(emb, pse, RELU, bias=bc_sb[:, 0:1])

        head_rows = [(0, 6), (6, 9), (15, 1)]
        for hidx, (r0, rn) in enumerate(head_rows):
            ps1 = pssc.tile([128, G], F32, tag="pssc")
            nc.tensor.matmul(ps1, lhsT=WH1_sb[:, hidx * HID:(hidx + 1) * HID],
                             rhs=emb, start=True, stop=True)
            th = small.tile([128, G], F32, tag="th")
            nc.scalar.activation(th, ps1, RELU, bias=BH1_sb[:, hidx:hidx + 1])
            ps2 = pstr.tile([N_HEAD_OUT, G], F32, tag="pstr")
            nc.tensor.matmul(ps2[0:rn, :], lhsT=WH2_sb[:, r0:r0 + rn],
                             rhs=th, start=True, stop=True)
            hb = small.tile([N_HEAD_OUT, G], F32, tag="headsb")
            nc.vector.tensor_scalar_add(hb[0:rn, :], ps2[0:rn, :],
                                        BH2_sb[0:rn, hidx:hidx + 1])
            nc.sync.dma_start(d_out.ap()[r0:r0 + rn, :], hb[0:rn, :])

    nc.compile()
    return nc


# --------------------------------------------------------------------------
# public entry point
# --------------------------------------------------------------------------

_CACHE = {}


def _pack_bh2(inputs):
    out = np.zeros((N_HEAD_OUT, 3), np.float32)
    for j, k in enumerate(("bd2", "br2", "bv2")):
        b = np.asarray(inputs[k], np.float32).reshape(-1)
        out[: b.shape[0], j] = b
    return out


def _make_in_maps(inputs, per_core, n_nodes, n_cores):
    n_per = n_nodes // n_cores
    n_win = _cdiv(n_per, WIN)
    n_pad = n_win * 128
    G = N_GRAPHS

    def f32(a):
        return np.ascontiguousarray(np.asarray(a, np.float32))

    node_features = f32(inputs["node_features"])
    batch = np.asarray(inputs["batch"], np.int64)
    gfeat = f32(inputs["global_features"])

    counts = np.bincount(batch, minlength=G)[:G]
    invc = (1.0 / np.maximum(counts, 1.0)).astype(np.float32)
    INVC = np.ascontiguousarray(np.tile(invc[None, :], (128, 1)))
    IOTA = np.ascontiguousarray(
        np.tile(np.arange(WIN, dtype=np.float32)[None, :], (128, 1)))
    edt = ml_dtypes.bfloat16 if EDGE_BF16 else np.float32
    IOTAE = np.ascontiguousarray(IOTA.astype(edt))
    IDENT = np.eye(128, dtype=np.float32)

    shared = dict(
        gT=f32(gfeat.T),
        invc=INVC,
        iota=IOTA,
        iotae=IOTAE,
        ident=IDENT,
        W_in=f32(inputs["W_in"]),
        b_in=f32(inputs["b_in"]).reshape(128, 1),
        WG1=f32(inputs["gin_W1"]),
        WG2=f32(inputs["gin_W2"]),
        B1=f32(np.asarray(inputs["gin_b1"]).T),
        B2=f32(np.asarray(inputs["gin_b2"]).T),
        Wg=f32(inputs["Wg"]),
        bg=f32(inputs["bg"]).reshape(128, 1),
        Wc=f32(inputs["W_comb"]),
        bc=f32(inputs["b_comb"]).reshape(128, 1),
        WH1=f32(np.stack([np.asarray(inputs["Wd1"]),
                          np.asarray(inputs["Wr1"]),
                          np.asarray(inputs["Wv1"])])),
        BH1=f32(np.stack([np.asarray(inputs["bd1"]),
                          np.asarray(inputs["br1"]),
                          np.asarray(inputs["bv1"])]).T),
        WH2=f32(np.concatenate([np.asarray(inputs["Wd2"]),
                                np.asarray(inputs["Wr2"]),
                                np.asarray(inputs["Wv2"])], axis=1)),
        BH2=_pack_bh2(inputs),
    )
    assert shared["WH2"].shape == (HID, N_HEAD_OUT)
    assert shared["BH2"].shape == (N_HEAD_OUT, 3)

    in_maps = []
    for c in range(n_cores):
        xT = np.zeros((NODE_DIM, n_pad), np.float32)
        xT[:, :n_per] = node_features[c * n_per:(c + 1) * n_per].T
        bf = np.full(n_pad, -1.0, np.float32)
        bf[:n_per] = batch[c * n_per:(c + 1) * n_per]
        BATCHF = np.ascontiguousarray(bf.reshape(n_win, 128).T)
        m = dict(shared)
        m.update(
            xT=np.ascontiguousarray(xT),
            batchf=BATCHF,
            idlo=per_core[c]["idlo"],
            idhi=per_core[c]["idhi"],
            doff=np.ascontiguousarray(per_core[c]["doff"].astype(edt)),
        )
        in_maps.append(m)
    return in_maps


def kernel(**inputs):
    global LAST_RESULT
    edge_index = np.asarray(inputs["edge_index"], np.int64)
    src, dst = edge_index[0], edge_index[1]
    n_nodes = int(np.asarray(inputs["node_features"]).shape[0])
    eps_vals = np.asarray(inputs["gin_eps"], np.float32)

    T, per_core = _preprocess_edges(src, dst, n_nodes, N_CORES, WIN, SPLIT)
    key = (n_nodes, N_CORES, WIN, CH, EDGE_BF16, NSWQ,
           tuple(int(x) for x in T),
           tuple(float(e) for e in eps_vals))
    if key not in _CACHE:
        _CACHE[key] = _build_nc(T, eps_vals, n_nodes, N_CORES)
    nc = _CACHE[key]

    in_maps = _make_in_maps(inputs, per_core, n_nodes, N_CORES)
    res = bass_utils.run_bass_kernel_spmd(
        nc, in_maps, core_ids=list(range(N_CORES)), trace=TRACE)
    LAST_RESULT = res
    heads = res.results[0]["heads"]  # [16, G]
    destroy = np.ascontiguousarray(heads[0:6].T)
    repair = np.ascontiguousarray(heads[6:15].T)
    value = np.ascontiguousarray(heads[15:16].T)
    return destroy, repair, value
